# revision 10
# baseline (speedup 1.0000x reference)
"""AWBNet (wo R2) Trainium2 kernel — v2.

Math (per sample b):
  m = reshape(relu(hist_flat @ W1 + b1) @ W2 + b2, [9, 3])
  feats(px) = [r, g, b, r^2, g^2, b^2, rg, rb, gb]
  y[px, c] = sum_k feats[px, k] * m[k, c]

Device strategy (8 cores, data parallel over batch, 2 samples/core):
  * Samples are split across SBUF partitions (sample 0 -> partitions 0..63,
    sample 1 -> 64..127), so one per-partition scalar vector carries the
    right m coefficients for every partition and ops span the full width.
  * Host packs x into fp16 channel planes (pure layout/dtype change), so
    the kernel never de-interleaves; W1 is streamed in fp16.
  * Tiny MLP on TensorE; m is broadcast per-partition by matmuls whose
    lhsT is a stride-0 broadcast column of featT (per 64-partition half).
  * Per-pixel combine y_c = sum_k m_kc B_k is split across engines:
      - 9 shared basis planes: r,g,b from DMA; squares on ACT; crosses on DVE
      - most (k,c) terms accumulate on the otherwise idle TensorE as
        diag(m_kc) @ B_k matmuls into PSUM (per-partition diagonals also
        encode the per-sample coefficients)
      - the rest form an SBUF fp16 partial on DVE (tensor_scalar products
        at 4x + adds) with some products on ACT; one extra identity matmul
        folds the partial into PSUM
      - ACT evicts PSUM -> fp16 SBUF, DMA writes planes out, host
        re-interleaves.
"""

import sys

import numpy as np

for _p in ("/opt/trn_rl_repo",):
    if _p not in sys.path:
        sys.path.insert(0, _p)

import concourse.bacc as bacc
import concourse.mybir as mybir
import concourse.tile as tile
from concourse import bass_utils

# ---- problem constants (hardcoded per contract) ----
N_CORES = 8
B, H, W, C = 16, 512, 512, 3
SPC = B // N_CORES  # samples per core = 2
PX_SAMPLE = H * W  # 262144
P = 128
PPS = P // SPC  # partitions per sample = 64
LANE_PX = PX_SAMPLE // PPS  # 4096 pixels per partition
NT = 2  # pixel tiles per core
TFD = LANE_PX // NT  # 2048 free-dim per tile
NCHUNK = TFD // 512  # 4 psum chunks of 512 per tile

HIST = 3 * 64 * 64  # 12288
HID = 256
MOUT = 27
KT = HIST // P  # 96 k-tiles
MT = HID // P  # 2 m-tiles
W1_CH = 8  # k-tiles per W1 DMA chunk

F16 = mybir.dt.float16
F32 = mybir.dt.float32
MULT = mybir.AluOpType.mult
ADD = mybir.AluOpType.add

# basis order: 0:r 1:g 2:b 3:r2 4:g2 5:b2 6:rg 7:rb 8:gb
# split of the 9 coefficients per channel across engines
PE_K = (0, 1, 2, 3, 4)  # accumulated on TensorE via diag matmuls
DVE_K = (5, 6, 7)  # tensor_scalar product on DVE
ACT_K = (8,)  # product on ACT, added on DVE

_CACHE = {}


def _build():
    nc = bacc.Bacc(
        "TRN2", target_bir_lowering=False, debug=False, num_devices=N_CORES
    )

    xp_d = nc.dram_tensor("xp", [NT, P, C, TFD], F16, kind="ExternalInput")
    w1_d = nc.dram_tensor("w1pm", [P, KT, HID], F16, kind="ExternalInput")
    hp_d = nc.dram_tensor("h_packed", [P, KT * SPC], F16, kind="ExternalInput")
    b1_d = nc.dram_tensor("b1_rep", [SPC, HID], F32, kind="ExternalInput")
    w2_d = nc.dram_tensor("w2p", [MT, P, MOUT], F32, kind="ExternalInput")
    b2_d = nc.dram_tensor("b2bc", [P, MOUT], F32, kind="ExternalInput")
    eye2_d = nc.dram_tensor("eye2", [SPC, SPC], F32, kind="ExternalInput")
    eyeP_d = nc.dram_tensor("eyeP", [P, P], F16, kind="ExternalInput")
    y_d = nc.dram_tensor("y_planes", [C, NT, P, TFD], F16, kind="ExternalOutput")

    n_pe = len(PE_K)

    with tile.TileContext(nc) as tc:
        with (
            tc.tile_pool(name="mlp", bufs=1) as mlp_pool,
            tc.tile_pool(name="w1s", bufs=1) as w1_pool,
            tc.tile_pool(name="px", bufs=1) as px_pool,
            tc.tile_pool(name="tmp", bufs=2) as tmp_pool,
        ):
            # ---------------- input DMAs ----------------
            # hp + first W1 chunks first (critical path to m); W1 stream is
            # split across the sync + gpsimd queues, x planes ride scalar.
            hp_sb = mlp_pool.tile([P, KT * SPC], F16, tag="hp", name="hp")
            nc.gpsimd.dma_start(out=hp_sb, in_=hp_d[:, :])

            # x tile 0 first on scalar (needed early for basis build)
            rgb = []
            r_0 = px_pool.tile([P, C, TFD], F16, tag="rgb0", name="rgb0")
            nc.scalar.dma_start(out=r_0, in_=xp_d[0])
            rgb.append(r_0)

            # W1 stream: 7 chunks on sync HWDGE, 5 on scalar (after x tile 0)
            NCHW1 = KT // W1_CH  # 12 W1 chunks, all resident (48KB/partition)
            w1_sbs = []
            for kc in range(NCHW1):
                w1_sb = w1_pool.tile([P, W1_CH, HID], F16, tag=f"w1c{kc}", name=f"w1c{kc}")
                q = nc.sync if kc < 7 else nc.scalar
                q.dma_start(out=w1_sb, in_=w1_d[:, kc * W1_CH : (kc + 1) * W1_CH, :])
                w1_sbs.append(w1_sb)

            # x tile 1 on the (slower) SWDGE queue; needed only mid-stream
            r_1 = px_pool.tile([P, C, TFD], F16, tag="rgb1", name="rgb1")
            nc.gpsimd.dma_start(out=r_1, in_=xp_d[1])
            rgb.append(r_1)

            # small setup DMAs on SWDGE
            b1_sb = mlp_pool.tile([SPC, HID], F32, tag="b1", name="b1")
            nc.gpsimd.dma_start(out=b1_sb, in_=b1_d[:, :])
            w2_sb = mlp_pool.tile([P, MT, MOUT], F32, tag="w2", name="w2")
            nc.gpsimd.dma_start(out=w2_sb, in_=w2_d.rearrange("m p n -> p m n"))
            b2_sb = mlp_pool.tile([P, MOUT], F32, tag="b2", name="b2")
            nc.gpsimd.dma_start(out=b2_sb, in_=b2_d[:, :])
            eye2_sb = mlp_pool.tile([SPC, SPC], F32, tag="eye2", name="eye2")
            nc.gpsimd.dma_start(out=eye2_sb, in_=eye2_d[:, :])
            eyeP_sb = mlp_pool.tile([P, P], F16, tag="eyeP", name="eyeP")
            nc.gpsimd.dma_start(out=eyeP_sb, in_=eyeP_d[:, :])

            # ---------------- MLP (TensorE) ----------------
            with tc.tile_pool(name="mlpps", bufs=1, space="PSUM") as mlp_psum:
                feat_ps = mlp_psum.tile([SPC, HID], F32, tag="featps", name="featps")
                for kc in range(NCHW1):
                    w1_sb = w1_sbs[kc]
                    for kk in range(W1_CH):
                        k = kc * W1_CH + kk
                        nc.tensor.matmul(
                            feat_ps,
                            hp_sb[:, k * SPC : (k + 1) * SPC],
                            w1_sb[:, kk, :],
                            start=(k == 0),
                            stop=(k == KT - 1),
                        )

                # relu(feat + b1) on DVE
                feat_sb = mlp_pool.tile([SPC, HID], F32, tag="featsb", name="featsb")
                nc.vector.tensor_add(feat_sb, feat_ps, b1_sb)
                feat_r = mlp_pool.tile([SPC, HID], F32, tag="featr", name="featr")
                nc.vector.tensor_scalar(
                    feat_r, feat_sb, 0.0, None, mybir.AluOpType.max
                )

                # transpose feat [2, 256] -> featT tiles [128, 2] via PE
                featT_sb = []
                for mt in range(MT):
                    ft_ps = mlp_psum.tile(
                        [P, SPC], F32, tag=f"ftps{mt}", name=f"ftps{mt}"
                    )
                    nc.tensor.transpose(
                        ft_ps, feat_r[:, mt * P : (mt + 1) * P], eye2_sb
                    )
                    ft_sb = mlp_pool.tile(
                        [P, SPC], F32, tag=f"ftsb{mt}", name=f"ftsb{mt}"
                    )
                    nc.vector.tensor_copy(ft_sb, ft_ps)
                    featT_sb.append(ft_sb)

                # m-matmul with per-half broadcast: partitions 64s..64s+63 get
                # sample s's coefficients.
                ms_ps = mlp_psum.tile([P, MOUT], F32, tag="msps", name="msps")
                for s in range(SPC):
                    for mt in range(MT):
                        nc.tensor.matmul(
                            ms_ps[s * PPS : (s + 1) * PPS, :],
                            featT_sb[mt][:, s : s + 1].broadcast_to([P, PPS]),
                            w2_sb[:, mt, :],
                            start=(mt == 0),
                            stop=(mt == MT - 1),
                        )
                mscal = mlp_pool.tile([P, MOUT], F32, tag="mscal", name="mscal")
                nc.vector.tensor_add(mscal, ms_ps, b2_sb)

            def ms(k, c):
                j = 3 * k + c
                return mscal[:, j : j + 1]

            # ---------------- shared basis planes ----------------
            # squares on ACT, crosses on DVE (overlaps the W1 stream)
            sq = []
            cross = []
            for t in range(NT):
                sq_t = px_pool.tile([P, C, TFD], F16, tag=f"sq{t}", name=f"sq{t}")
                nc.scalar.square(sq_t, rgb[t])
                sq.append(sq_t)
                cr_t = px_pool.tile([P, C, TFD], F16, tag=f"cr{t}", name=f"cr{t}")
                nc.vector.tensor_mul(cr_t[:, 0, :], rgb[t][:, 0, :], rgb[t][:, 1, :])
                nc.vector.tensor_mul(cr_t[:, 1, :], rgb[t][:, 0, :], rgb[t][:, 2, :])
                nc.vector.tensor_mul(cr_t[:, 2, :], rgb[t][:, 1, :], rgb[t][:, 2, :])
                cross.append(cr_t)

            def basis(k, t):
                if k < 3:
                    return rgb[t][:, k, :]
                if k < 6:
                    return sq[t][:, k - 3, :]
                return cross[t][:, k - 6, :]

            # diag(m_kc) weights for the PE-accumulated terms, built
            # just-in-time per channel on ACT (DVE is the storm bottleneck)
            diags = mlp_pool.tile([P, C, n_pe, P], F16, tag="diags", name="diags")

            # ---------------- per-(channel, tile) combine ----------------
            with tc.tile_pool(name="pxps", bufs=2, space="PSUM") as px_psum:
                for c in range(C):
                    for i, k in enumerate(PE_K):
                        nc.scalar.mul(diags[:, c, i, :], eyeP_sb, ms(k, c))
                    for t in range(NT):
                        # DVE/ACT partial: sum over DVE_K + ACT_K
                        part = tmp_pool.tile([P, TFD], F16, tag="part", name=f"pt{t}{c}")
                        k0 = DVE_K[0]
                        nc.vector.tensor_scalar(
                            part, basis(k0, t), ms(k0, c), None, MULT
                        )
                        for k in DVE_K[1:]:
                            u = tmp_pool.tile([P, TFD], F16, tag="u", name=f"u{t}{c}{k}")
                            nc.vector.tensor_scalar(u, basis(k, t), ms(k, c), None, MULT)
                            nc.vector.tensor_add(part, part, u)
                        for k in ACT_K:
                            v = tmp_pool.tile([P, TFD], F16, tag="v", name=f"v{t}{c}{k}")
                            nc.scalar.mul(v, basis(k, t), ms(k, c))
                            nc.vector.tensor_add(part, part, v)

                        # PE accumulation into PSUM (k outer: consecutive
                        # matmuls share the stationary diag -> LDW amortized)
                        yc_ps = px_psum.tile([P, TFD], F32, tag="yc", name=f"yc{t}{c}")
                        for i, k in enumerate(PE_K):
                            for n in range(NCHUNK):
                                sl = slice(n * 512, (n + 1) * 512)
                                nc.tensor.matmul(
                                    yc_ps[:, sl],
                                    diags[:, c, i, :],
                                    basis(k, t)[:, sl],
                                    start=(i == 0),
                                    stop=False,
                                )
                        for n in range(NCHUNK):
                            sl = slice(n * 512, (n + 1) * 512)
                            nc.tensor.matmul(
                                yc_ps[:, sl],
                                eyeP_sb,
                                part[:, sl],
                                start=False,
                                stop=True,
                            )

                        # evict PSUM -> fp16 SBUF on ACT, then DMA out
                        y_sb = tmp_pool.tile([P, TFD], F16, tag="ysb", name=f"y{t}{c}")
                        nc.scalar.copy(y_sb, yc_ps)
                        nc.sync.dma_start(out=y_d[c, t], in_=y_sb)

    nc.compile()
    return nc


def _prep_inputs(x, histogram, W1, b1, W2, b2):
    """Host-side sharding / layout packing (layout + dtype only; no data
    arithmetic)."""
    x = np.asarray(x, dtype=np.float32)
    hist = np.asarray(histogram, dtype=np.float32).reshape(B, HIST)
    W1 = np.asarray(W1, dtype=np.float32)
    b1 = np.asarray(b1, dtype=np.float32)
    W2 = np.asarray(W2, dtype=np.float32)
    b2 = np.asarray(b2, dtype=np.float32)

    # W1 partition-major fp16: w1pm[p, k, :] = W1[k*128 + p, :]
    w1pm = np.ascontiguousarray(
        W1.reshape(KT, P, HID).transpose(1, 0, 2)
    ).astype(np.float16)
    w2p = np.ascontiguousarray(W2.reshape(MT, P, MOUT))
    b1rep = np.ascontiguousarray(np.broadcast_to(b1, (SPC, HID)))
    b2bc = np.ascontiguousarray(np.broadcast_to(b2, (P, MOUT)))
    eye2 = np.eye(SPC, dtype=np.float32)
    eyeP = np.eye(P, dtype=np.float16)

    in_maps = []
    for core in range(N_CORES):
        # x planes: [NT, P, C, TFD] fp16, sample s on partitions 64s..64s+63
        xp = np.empty((NT, P, C, TFD), dtype=np.float16)
        for s in range(SPC):
            xs = x[core * SPC + s].reshape(PX_SAMPLE, C)
            # pixel index = p*LANE_PX + t*TFD + q
            v = xs.reshape(PPS, NT, TFD, C).transpose(1, 0, 3, 2)  # [t, p, c, q]
            xp[:, s * PPS : (s + 1) * PPS, :, :] = v.astype(np.float16)

        h_core = hist[core * SPC : (core + 1) * SPC]  # [SPC, HIST]
        hp = np.ascontiguousarray(
            h_core.reshape(SPC, KT, P).transpose(2, 1, 0).reshape(P, KT * SPC)
        ).astype(np.float16)
        in_maps.append(
            {
                "xp": xp,
                "w1pm": w1pm,
                "h_packed": hp,
                "b1_rep": b1rep,
                "w2p": w2p,
                "b2bc": b2bc,
                "eye2": eye2,
                "eyeP": eyeP,
            }
        )
    return in_maps


def _unpack_output(res):
    y = np.empty((B, H, W, C), dtype=np.float32)
    for core in range(N_CORES):
        planes = np.asarray(res.results[core]["y_planes"])  # [C, NT, P, TFD] f16
        for s in range(SPC):
            v = planes[:, :, s * PPS : (s + 1) * PPS, :]  # [C, NT, PPS, TFD]
            v = v.transpose(2, 1, 3, 0).reshape(PX_SAMPLE, C)  # px=(p,t,q)
            y[core * SPC + s] = v.reshape(H, W, C).astype(np.float32)
    return y


def run(trace=False, **inputs):
    if "nc" not in _CACHE:
        _CACHE["nc"] = _build()
    nc = _CACHE["nc"]
    in_maps = _prep_inputs(**inputs)
    res = bass_utils.run_bass_kernel_spmd(
        nc, in_maps, core_ids=list(range(N_CORES)), trace=trace
    )
    y = _unpack_output(res)
    return y, res


def kernel(**inputs) -> np.ndarray:
    y, _ = run(trace=False, **inputs)
    return y


if __name__ == "__main__":
    rng = np.random.default_rng(0)
    ins = {
        "x": rng.random((B, H, W, C), dtype=np.float32),
        "histogram": rng.random((B, 3, 64, 64), dtype=np.float32),
        "W1": (rng.standard_normal((HIST, HID)) / np.sqrt(HIST)).astype(np.float32),
        "b1": np.zeros(HID, np.float32),
        "W2": (rng.standard_normal((HID, MOUT)) / np.sqrt(HID)).astype(np.float32),
        "b2": np.zeros(MOUT, np.float32),
    }
    y = kernel(**ins)
    print("out", y.shape, y.dtype, float(np.abs(y).max()))


# revision 14
# speedup vs baseline: 1.0782x; 1.0782x over previous
"""AWBNet (wo R2) Trainium2 kernel — v5 (interleaved band layout).

Math (per sample b):
  m = reshape(relu(hist_flat @ W1 + b1) @ W2 + b2, [9, 3])
  feats(px) = [r, g, b, r^2, g^2, b^2, rg, rb, gb]
  y[px, c] = sum_k feats[px, k] * m[k, c]

Device strategy (8 cores, data parallel over batch, 2 samples/core):
  * SBUF partitions hold (band i, group g): 3 bands x 42 pixel groups = 126
    partitions; groups 0..20 belong to sample 0, 21..41 to sample 1. The
    host ships x as xI[42i+g, n] = x_i(pixel (g, n)) plus a band-rotated
    copy xR (pure layout duplication) so rg/gb/br products are
    lane-aligned.
  * Per-pixel combine is THREE block-diagonal matmuls per column chunk:
    out[42c+g, n] += sum_i M_mat[i, c; s(g)] * plane_mat[42i+g, n] for
    plane in {xI (linear), xI^2 (squares, one DVE op), xI*xR (crosses,
    one DVE op)}. The [3,3] diagonal blocks carry per-sample
    coefficients, built from the MLP output with one masked
    tensor_scalar per (mat, c).
  * Tiny MLP on TensorE (fp16 W1 streamed over two DMA queues); ACT
    evicts PSUM -> fp16 planes; host re-interleaves.
"""

import sys

import numpy as np

for _p in ("/opt/trn_rl_repo",):
    if _p not in sys.path:
        sys.path.insert(0, _p)

import concourse.bacc as bacc
import concourse.mybir as mybir
import concourse.tile as tile
from concourse import bass_utils

# ---- problem constants (hardcoded per contract) ----
N_CORES = 8
B, H, W, C = 16, 512, 512, 3
SPC = B // N_CORES  # samples per core = 2
PX_SAMPLE = H * W  # 262144
P = 128

G_S = 21  # pixel groups per sample
G = SPC * G_S  # 42 groups
NP = 3 * G  # 126 used partitions
XCOLS = 12800  # padded pixels per group (21*12800 >= 262144)
NSTAGE = 7  # psum stages: 6 x 2048 + 1 x 512
STAGE_COLS = (2048, 2048, 2048, 2048, 2048, 2048, 512)

HIST = 3 * 64 * 64  # 12288
HID = 256
MOUT = 27
KT = HIST // P  # 96 k-tiles
MT = HID // P  # 2 m-tiles
W1_CH = 8  # k-tiles per W1 DMA chunk

F16 = mybir.dt.float16
F32 = mybir.dt.float32
MULT = mybir.AluOpType.mult

_CACHE = {}


def _colmap(mat, i, c):
    """W2/b2 column for (matrix, band, out-channel): which of the 27
    m-coefficients scales plane_mat band i into channel c."""
    if mat == 0:  # linear: x_i
        k = i
    elif mat == 1:  # squares: x_i^2
        k = 3 + i
    else:  # crosses: x_i * x_{(i+1)%3} -> rg, gb, br
        k = (6, 8, 7)[i]
    return 3 * k + c


def _build():
    nc = bacc.Bacc(
        "TRN2", target_bir_lowering=False, debug=False, num_devices=N_CORES
    )

    xi_d = nc.dram_tensor("xi", [NP, XCOLS], F16, kind="ExternalInput")
    xr_d = nc.dram_tensor("xr", [NP, XCOLS], F16, kind="ExternalInput")
    w1_d = nc.dram_tensor("w1pm", [P, KT, HID], F16, kind="ExternalInput")
    hp_d = nc.dram_tensor("h_packed", [P, KT * SPC], F16, kind="ExternalInput")
    b1_d = nc.dram_tensor("b1_rep", [SPC, HID], F32, kind="ExternalInput")
    w2_d = nc.dram_tensor("w2i", [MT, P, 3 * 9], F32, kind="ExternalInput")
    b2_d = nc.dram_tensor("b2i", [NP, 27], F32, kind="ExternalInput")
    eye2_d = nc.dram_tensor("eye2", [SPC, SPC], F32, kind="ExternalInput")
    mask_d = nc.dram_tensor("maskS", [NP, G], F16, kind="ExternalInput")
    y_d = nc.dram_tensor("y_bands", [NP, XCOLS], F16, kind="ExternalOutput")

    with tile.TileContext(nc) as tc:
        with (
            tc.tile_pool(name="mlp", bufs=1) as mlp_pool,
            tc.tile_pool(name="w1s", bufs=1) as w1_pool,
            tc.tile_pool(name="px", bufs=1) as px_pool,
            tc.tile_pool(name="ring", bufs=2) as ring_pool,
        ):
            # ---------------- input DMAs ----------------
            hp_sb = mlp_pool.tile([P, KT * SPC], F16, tag="hp", name="hp")
            nc.gpsimd.dma_start(out=hp_sb, in_=hp_d[:, :])

            # x first half on scalar (needed for early storm stages)
            XH = 6144
            xi_sb = px_pool.tile([NP, XCOLS], F16, tag="xi", name="xi")
            nc.scalar.dma_start(out=xi_sb[:, 0:XH], in_=xi_d[:, 0:XH])

            # W1 stream: 8 chunks on sync, 4 on scalar
            NCHW1 = KT // W1_CH  # 12
            w1_sbs = []
            for kc in range(NCHW1):
                w1_sb = w1_pool.tile(
                    [P, W1_CH, HID], F16, tag=f"w1c{kc}", name=f"w1c{kc}"
                )
                q = nc.sync if kc % 3 != 2 else nc.scalar
                q.dma_start(out=w1_sb, in_=w1_d[:, kc * W1_CH : (kc + 1) * W1_CH, :])
                w1_sbs.append(w1_sb)

            # remaining x / xR (ordered after W1 on scalar; xR0 before x1
            # so early stages get their cross factor)
            xr_sb = px_pool.tile([NP, XCOLS], F16, tag="xr", name="xr")
            nc.scalar.dma_start(out=xr_sb[:, 0:XH], in_=xr_d[:, 0:XH])
            nc.scalar.dma_start(out=xi_sb[:, XH:XCOLS], in_=xi_d[:, XH:XCOLS])
            nc.scalar.dma_start(out=xr_sb[:, XH:XCOLS], in_=xr_d[:, XH:XCOLS])

            # small setup DMAs on SWDGE
            b1_sb = mlp_pool.tile([SPC, HID], F32, tag="b1", name="b1")
            nc.gpsimd.dma_start(out=b1_sb, in_=b1_d[:, :])
            w2_sb = mlp_pool.tile([P, MT, 3 * 9], F32, tag="w2", name="w2")
            nc.gpsimd.dma_start(out=w2_sb, in_=w2_d.rearrange("m p n -> p m n"))
            b2_sb = mlp_pool.tile([NP, 27], F32, tag="b2", name="b2")
            nc.gpsimd.dma_start(out=b2_sb, in_=b2_d[:, :])
            eye2_sb = mlp_pool.tile([SPC, SPC], F32, tag="eye2", name="eye2")
            nc.gpsimd.dma_start(out=eye2_sb, in_=eye2_d[:, :])
            mask_sb = mlp_pool.tile([NP, G], F16, tag="mask", name="mask")
            nc.gpsimd.dma_start(out=mask_sb, in_=mask_d[:, :])

            # ---------------- MLP (TensorE) ----------------
            with tc.tile_pool(name="mlpps", bufs=1, space="PSUM") as mlp_psum:
                feat_ps = mlp_psum.tile([SPC, HID], F32, tag="featps", name="featps")
                for kc in range(NCHW1):
                    w1_sb = w1_sbs[kc]
                    for kk in range(W1_CH):
                        k = kc * W1_CH + kk
                        nc.tensor.matmul(
                            feat_ps,
                            hp_sb[:, k * SPC : (k + 1) * SPC],
                            w1_sb[:, kk, :],
                            start=(k == 0),
                            stop=(k == KT - 1),
                        )

                feat_sb = mlp_pool.tile([SPC, HID], F32, tag="featsb", name="featsb")
                nc.vector.tensor_add(feat_sb, feat_ps, b1_sb)
                feat_r = mlp_pool.tile([SPC, HID], F32, tag="featr", name="featr")
                nc.vector.tensor_scalar(
                    feat_r, feat_sb, 0.0, None, mybir.AluOpType.max
                )

                featT_sb = []
                for mt in range(MT):
                    ft_ps = mlp_psum.tile(
                        [P, SPC], F32, tag=f"ftps{mt}", name=f"ftps{mt}"
                    )
                    nc.tensor.transpose(
                        ft_ps, feat_r[:, mt * P : (mt + 1) * P], eye2_sb
                    )
                    ft_sb = mlp_pool.tile(
                        [P, SPC], F32, tag=f"ftsb{mt}", name=f"ftsb{mt}"
                    )
                    nc.vector.tensor_copy(ft_sb, ft_ps)
                    featT_sb.append(ft_sb)

                # msP27[42i+21s+g', 27-col] = all m-coefficients for sample
                # s (band-grouped column order). lhsT = sample-pattern tile
                # (feat column s(p') replicated per region) so the single
                # matmul writes all 126 partitions at base 0.
                msP_ps = mlp_psum.tile([NP, 27], F32, tag="msps", name="msps")
                for mt in range(MT):
                    patt = mlp_pool.tile([P, NP], F32, tag=f"patt{mt}", name=f"patt{mt}")
                    for i in range(3):
                        for s in range(SPC):
                            p0 = 42 * i + G_S * s
                            nc.vector.tensor_copy(
                                patt[:, p0 : p0 + G_S],
                                featT_sb[mt][:, s : s + 1].broadcast_to([P, G_S]),
                            )
                    nc.tensor.matmul(
                        msP_ps,
                        patt,
                        w2_sb[:, mt, :],
                        start=(mt == 0),
                        stop=(mt == MT - 1),
                    )
                msP = mlp_pool.tile([NP, 27], F32, tag="msP", name="msP")
                nc.vector.tensor_add(msP, msP_ps, b2_sb)

            # gather each band's 9 relevant columns (engine ops cannot start
            # at partition 42, DMAs can) -> msP9[p, 3*mat+c]
            msP9 = mlp_pool.tile([NP, 9], F32, tag="msP9", name="msP9")
            for i in range(3):
                bs = slice(42 * i, 42 * (i + 1))
                nc.gpsimd.dma_start(
                    out=msP9[bs, :], in_=msP[bs, 9 * i : 9 * (i + 1)]
                )

            # block-diagonal weight matrices: one masked tensor_scalar per
            # (mat, c) writes column band c of lhsT_mat (full width)
            lhsT = mlp_pool.tile([NP, 3, NP], F16, tag="lhsT", name="lhsT")
            for mat in range(3):
                for c in range(C):
                    nc.vector.tensor_scalar(
                        lhsT[:, mat, G * c : G * (c + 1)],
                        mask_sb,
                        msP9[:, 3 * mat + c : 3 * mat + c + 1],
                        None,
                        MULT,
                    )

            # ---------------- storm: per-stage basis + 3 matmuls ----------
            with tc.tile_pool(name="pxps", bufs=2, space="PSUM") as px_psum:
                col0 = 0
                for st in range(NSTAGE):
                    ncols = STAGE_COLS[st]
                    sl = slice(col0, col0 + ncols)

                    sq_t = ring_pool.tile([NP, 2048], F16, tag="sq", name=f"sq{st}")
                    nc.vector.tensor_mul(
                        sq_t[:, 0:ncols], xi_sb[:, sl], xi_sb[:, sl]
                    )
                    cr_t = ring_pool.tile([NP, 2048], F16, tag="cr", name=f"cr{st}")
                    nc.vector.tensor_mul(
                        cr_t[:, 0:ncols], xi_sb[:, sl], xr_sb[:, sl]
                    )

                    yc_ps = px_psum.tile([NP, 2048], F32, tag="yc", name=f"yc{st}")
                    nch = (ncols + 511) // 512
                    for mat in range(3):
                        for n in range(nch):
                            c0 = n * 512
                            c1 = min(c0 + 512, ncols)
                            if mat == 0:
                                rhs = xi_sb[:, col0 + c0 : col0 + c1]
                            elif mat == 1:
                                rhs = sq_t[:, c0:c1]
                            else:
                                rhs = cr_t[:, c0:c1]
                            nc.tensor.matmul(
                                yc_ps[:, c0:c1],
                                lhsT[:, mat, :],
                                rhs,
                                start=(mat == 0),
                                stop=(mat == 2),
                            )

                    y_sb = ring_pool.tile([NP, 2048], F16, tag="ysb", name=f"y{st}")
                    nc.scalar.copy(y_sb[:, 0:ncols], yc_ps[:, 0:ncols])
                    nc.sync.dma_start(out=y_d[:, sl], in_=y_sb[:, 0:ncols])
                    col0 += ncols

    nc.compile()
    return nc


def _prep_inputs(x, histogram, W1, b1, W2, b2):
    """Host-side sharding / layout packing (layout + dtype only; no data
    arithmetic)."""
    x = np.asarray(x, dtype=np.float32)
    hist = np.asarray(histogram, dtype=np.float32).reshape(B, HIST)
    W1 = np.asarray(W1, dtype=np.float32)
    b1 = np.asarray(b1, dtype=np.float32)
    W2 = np.asarray(W2, dtype=np.float32)
    b2 = np.asarray(b2, dtype=np.float32)

    w1pm = np.ascontiguousarray(
        W1.reshape(KT, P, HID).transpose(1, 0, 2)
    ).astype(np.float16)
    b1rep = np.ascontiguousarray(np.broadcast_to(b1, (SPC, HID)))
    eye2 = np.eye(SPC, dtype=np.float32)

    # W2 / b2 with interleave-mapped columns
    cm = np.empty((3, 9), dtype=np.int64)  # [i, 3*mat+c]
    for i in range(3):
        for mat in range(3):
            for c in range(C):
                cm[i, 3 * mat + c] = _colmap(mat, i, c)
    w2i = np.ascontiguousarray(
        W2.reshape(MT, P, MOUT)[:, :, cm.reshape(-1)].reshape(MT, P, 3, 9)
        .reshape(MT, P, 27)
    )
    b2i = np.ascontiguousarray(
        np.broadcast_to(b2[cm.reshape(-1)], (NP, 27))
    ).astype(np.float32)

    maskS = np.zeros((NP, G), dtype=np.float16)
    for i in range(3):
        for g in range(G):
            maskS[42 * i + g, g] = 1.0

    rot = np.concatenate(
        [np.arange(42, 84), np.arange(84, 126), np.arange(0, 42)]
    )

    in_maps = []
    for core in range(N_CORES):
        xI = np.zeros((NP, XCOLS), dtype=np.float16)
        for s in range(SPC):
            xs = x[core * SPC + s].reshape(PX_SAMPLE, C)
            pad = np.zeros((G_S * XCOLS, C), dtype=np.float32)
            pad[:PX_SAMPLE] = xs
            v = pad.reshape(G_S, XCOLS, C)  # [g', n, i]
            for i in range(3):
                xI[42 * i + G_S * s : 42 * i + G_S * (s + 1), :] = v[:, :, i].astype(
                    np.float16
                )
        xR = np.ascontiguousarray(xI[rot])

        h_core = hist[core * SPC : (core + 1) * SPC]
        hp = np.ascontiguousarray(
            h_core.reshape(SPC, KT, P).transpose(2, 1, 0).reshape(P, KT * SPC)
        ).astype(np.float16)
        in_maps.append(
            {
                "xi": xI,
                "xr": xR,
                "w1pm": w1pm,
                "h_packed": hp,
                "b1_rep": b1rep,
                "w2i": w2i,
                "b2i": b2i,
                "eye2": eye2,
                "maskS": maskS,
            }
        )
    return in_maps


def _unpack_output(res):
    y = np.empty((B, H, W, C), dtype=np.float32)
    for core in range(N_CORES):
        yb = np.asarray(res.results[core]["y_bands"])  # [126, XCOLS] f16
        for s in range(SPC):
            v = yb[:, :].reshape(3, G, XCOLS)[:, G_S * s : G_S * (s + 1), :]
            # v[c, g', n] -> pixel g'*XCOLS + n
            flat = v.transpose(1, 2, 0).reshape(G_S * XCOLS, C)[:PX_SAMPLE]
            y[core * SPC + s] = flat.reshape(H, W, C).astype(np.float32)
    return y


def run(trace=False, **inputs):
    if "nc" not in _CACHE:
        _CACHE["nc"] = _build()
    nc = _CACHE["nc"]
    in_maps = _prep_inputs(**inputs)
    res = bass_utils.run_bass_kernel_spmd(
        nc, in_maps, core_ids=list(range(N_CORES)), trace=trace
    )
    y = _unpack_output(res)
    return y, res


def kernel(**inputs) -> np.ndarray:
    y, _ = run(trace=False, **inputs)
    return y


if __name__ == "__main__":
    rng = np.random.default_rng(0)
    ins = {
        "x": rng.random((B, H, W, C), dtype=np.float32),
        "histogram": rng.random((B, 3, 64, 64), dtype=np.float32),
        "W1": (rng.standard_normal((HIST, HID)) / np.sqrt(HIST)).astype(np.float32),
        "b1": np.zeros(HID, np.float32),
        "W2": (rng.standard_normal((HID, MOUT)) / np.sqrt(HID)).astype(np.float32),
        "b2": np.zeros(MOUT, np.float32),
    }
    y = kernel(**ins)
    print("out", y.shape, y.dtype, float(np.abs(y).max()))


# revision 17
# speedup vs baseline: 1.3544x; 1.2562x over previous
"""AWBNet (wo R2) Trainium2 kernel — v5 (interleaved band layout).

Math (per sample b):
  m = reshape(relu(hist_flat @ W1 + b1) @ W2 + b2, [9, 3])
  feats(px) = [r, g, b, r^2, g^2, b^2, rg, rb, gb]
  y[px, c] = sum_k feats[px, k] * m[k, c]

Device strategy (8 cores, data parallel over batch, 2 samples/core):
  * SBUF partitions hold (band i, group g): 3 bands x 42 pixel groups = 126
    partitions; groups 0..20 belong to sample 0, 21..41 to sample 1. The
    host ships x as xI[42i+g, n] = x_i(pixel (g, n)) plus a band-rotated
    copy xR (pure layout duplication) so rg/gb/br products are
    lane-aligned.
  * Per-pixel combine is THREE block-diagonal matmuls per column chunk:
    out[42c+g, n] += sum_i M_mat[i, c; s(g)] * plane_mat[42i+g, n] for
    plane in {xI (linear), xI^2 (squares, one DVE op), xI*xR (crosses,
    one DVE op)}. The [3,3] diagonal blocks carry per-sample
    coefficients, built from the MLP output with one masked
    tensor_scalar per (mat, c).
  * Tiny MLP on TensorE (fp16 W1 streamed over two DMA queues); ACT
    evicts PSUM -> fp16 planes; host re-interleaves.
"""

import sys

import numpy as np

for _p in ("/opt/trn_rl_repo",):
    if _p not in sys.path:
        sys.path.insert(0, _p)

import concourse.bacc as bacc
import concourse.mybir as mybir
import concourse.tile as tile
from concourse import bass_utils

# ---- problem constants (hardcoded per contract) ----
N_CORES = 8
B, H, W, C = 16, 512, 512, 3
SPC = B // N_CORES  # samples per core = 2
PX_SAMPLE = H * W  # 262144
P = 128

G_S = 21  # pixel groups per sample
G = SPC * G_S  # 42 groups
NP = 3 * G  # 126 used partitions
XCOLS = 12800  # padded pixels per group (21*12800 >= 262144)
NSTAGE = 7  # psum stages: 6 x 2048 + 1 x 512
STAGE_COLS = (2048, 2048, 2048, 2048, 2048, 2048, 512)

HIST = 3 * 64 * 64  # 12288
HID = 256
MOUT = 27
KT = HIST // P  # 96 k-tiles
MT = HID // P  # 2 m-tiles
W1_CH = 8  # k-tiles per W1 DMA chunk

F16 = mybir.dt.float16
F32 = mybir.dt.float32
MULT = mybir.AluOpType.mult

_CACHE = {}


def _colmap(mat, i, c):
    """W2/b2 column for (matrix, band, out-channel): which of the 27
    m-coefficients scales plane_mat band i into channel c."""
    if mat == 0:  # linear: x_i
        k = i
    elif mat == 1:  # squares: x_i^2
        k = 3 + i
    else:  # crosses: x_i * x_{(i+1)%3} -> rg, gb, br
        k = (6, 8, 7)[i]
    return 3 * k + c


def _build():
    nc = bacc.Bacc(
        "TRN2", target_bir_lowering=False, debug=False, num_devices=N_CORES
    )

    xi_d = nc.dram_tensor("xi", [NP, XCOLS], F16, kind="ExternalInput")
    xr_d = nc.dram_tensor("xr", [NP, XCOLS], F16, kind="ExternalInput")
    w1_d = nc.dram_tensor("w1pm", [P, KT, HID], F16, kind="ExternalInput")
    hp_d = nc.dram_tensor("h_packed", [P, KT * SPC], F16, kind="ExternalInput")
    b1_d = nc.dram_tensor("b1_rep", [SPC, HID], F32, kind="ExternalInput")
    w2_d = nc.dram_tensor("w2i", [MT, P, 3 * 9], F32, kind="ExternalInput")
    b2_d = nc.dram_tensor("b2i", [NP, 9], F32, kind="ExternalInput")
    e3_d = nc.dram_tensor("e3", [SPC, 3, NP], F32, kind="ExternalInput")
    mask_d = nc.dram_tensor("maskS", [NP, G], F16, kind="ExternalInput")
    y_d = nc.dram_tensor("y_bands", [NP, XCOLS], F16, kind="ExternalOutput")

    with tile.TileContext(nc) as tc:
        with (
            tc.tile_pool(name="mlp", bufs=1) as mlp_pool,
            tc.tile_pool(name="w1s", bufs=1) as w1_pool,
            tc.tile_pool(name="px", bufs=1) as px_pool,
            tc.tile_pool(name="ring", bufs=2) as ring_pool,
        ):
            # ---------------- input DMAs ----------------
            hp_sb = mlp_pool.tile([P, KT * SPC], F16, tag="hp", name="hp")
            nc.gpsimd.dma_start(out=hp_sb, in_=hp_d[:, :])

            # W1 stream has strict priority: 12 chunks alternate sync/scalar
            # in k-order so arrival order matches MM consumption.
            NCHW1 = KT // W1_CH  # 12
            w1_sbs = []
            for kc in range(NCHW1):
                w1_sb = w1_pool.tile(
                    [P, W1_CH, HID], F16, tag=f"w1c{kc}", name=f"w1c{kc}"
                )
                q = nc.sync if kc % 2 == 0 else nc.scalar
                q.dma_start(out=w1_sb, in_=w1_d[:, kc * W1_CH : (kc + 1) * W1_CH, :])
                w1_sbs.append(w1_sb)

            # x / xR stream per storm stage (xI on sync, xR on scalar), so
            # the storm chases the DMA with stage granularity.
            xi_sb = px_pool.tile([NP, XCOLS], F16, tag="xi", name="xi")
            xr_sb = px_pool.tile([NP, XCOLS], F16, tag="xr", name="xr")
            col0 = 0
            for st in range(NSTAGE):
                sl = slice(col0, col0 + STAGE_COLS[st])
                nc.sync.dma_start(out=xi_sb[:, sl], in_=xi_d[:, sl])
                nc.scalar.dma_start(out=xr_sb[:, sl], in_=xr_d[:, sl])
                col0 += STAGE_COLS[st]

            # small setup DMAs on SWDGE
            b1_sb = mlp_pool.tile([SPC, HID], F32, tag="b1", name="b1")
            nc.gpsimd.dma_start(out=b1_sb, in_=b1_d[:, :])
            w2_sb = mlp_pool.tile([P, MT, 3 * 9], F32, tag="w2", name="w2")
            nc.gpsimd.dma_start(out=w2_sb, in_=w2_d.rearrange("m p n -> p m n"))
            b2_sb = mlp_pool.tile([NP, 9], F32, tag="b2", name="b2")
            nc.gpsimd.dma_start(out=b2_sb, in_=b2_d[:, :])
            e3_sb = mlp_pool.tile([SPC, 3, NP], F32, tag="e3", name="e3")
            nc.gpsimd.dma_start(out=e3_sb, in_=e3_d[:, :, :])
            mask_sb = mlp_pool.tile([NP, G], F16, tag="mask", name="mask")
            nc.gpsimd.dma_start(out=mask_sb, in_=mask_d[:, :])

            # ---------------- MLP (TensorE) ----------------
            with tc.tile_pool(name="mlpps", bufs=1, space="PSUM") as mlp_psum:
                feat_ps = mlp_psum.tile([SPC, HID], F32, tag="featps", name="featps")
                for kc in range(NCHW1):
                    w1_sb = w1_sbs[kc]
                    for kk in range(W1_CH):
                        k = kc * W1_CH + kk
                        nc.tensor.matmul(
                            feat_ps,
                            hp_sb[:, k * SPC : (k + 1) * SPC],
                            w1_sb[:, kk, :],
                            start=(k == 0),
                            stop=(k == KT - 1),
                        )

                feat_sb = mlp_pool.tile([SPC, HID], F32, tag="featsb", name="featsb")
                nc.vector.tensor_add(feat_sb, feat_ps, b1_sb)
                feat_r = mlp_pool.tile([SPC, HID], F32, tag="featr", name="featr")
                nc.vector.tensor_scalar(
                    feat_r, feat_sb, 0.0, None, mybir.AluOpType.max
                )

                # msP9[42i+21s+g', 3*mat+c] = band-selected m-coefficients.
                # patt_i = feat_slice^T @ E_i (E_i = 0/1 selector, zero
                # outside band i) -> band matmuls accumulate over (mt, i).
                msP_ps = mlp_psum.tile([NP, 9], F32, tag="msps", name="msps")
                nmm = 0
                for mt in range(MT):
                    for i in range(3):
                        pt_ps = mlp_psum.tile(
                            [P, NP], F32, tag=f"ptps{mt}{i}", name=f"ptps{mt}{i}"
                        )
                        nc.tensor.matmul(
                            pt_ps,
                            feat_r[:, mt * P : (mt + 1) * P],
                            e3_sb[:, i, :],
                            start=True,
                            stop=True,
                        )
                        patt = mlp_pool.tile(
                            [P, NP], F32, tag=f"patt{mt}{i}", name=f"patt{mt}{i}"
                        )
                        nc.vector.tensor_copy(patt, pt_ps)
                        nc.tensor.matmul(
                            msP_ps,
                            patt,
                            w2_sb[:, mt, 9 * i : 9 * (i + 1)],
                            start=(nmm == 0),
                            stop=(nmm == 2 * MT + 1),
                        )
                        nmm += 1
                msP9 = mlp_pool.tile([NP, 9], F32, tag="msP9", name="msP9")
                nc.vector.tensor_add(msP9, msP_ps, b2_sb)

            # block-diagonal weight matrices: one masked tensor_scalar per
            # (mat, c) writes column band c of lhsT_mat (full width)
            lhsT = mlp_pool.tile([NP, 3, NP], F16, tag="lhsT", name="lhsT")
            for mat in range(3):
                for c in range(C):
                    nc.vector.tensor_scalar(
                        lhsT[:, mat, G * c : G * (c + 1)],
                        mask_sb,
                        msP9[:, 3 * mat + c : 3 * mat + c + 1],
                        None,
                        MULT,
                    )

            # ---------------- storm: per-stage basis + 3 matmuls ----------
            with tc.tile_pool(name="pxps", bufs=2, space="PSUM") as px_psum:
                col0 = 0
                for st in range(NSTAGE):
                    ncols = STAGE_COLS[st]
                    sl = slice(col0, col0 + ncols)

                    sq_t = ring_pool.tile([NP, 2048], F16, tag="sq", name=f"sq{st}")
                    nc.vector.tensor_mul(
                        sq_t[:, 0:ncols], xi_sb[:, sl], xi_sb[:, sl]
                    )
                    cr_t = ring_pool.tile([NP, 2048], F16, tag="cr", name=f"cr{st}")
                    nc.vector.tensor_mul(
                        cr_t[:, 0:ncols], xi_sb[:, sl], xr_sb[:, sl]
                    )

                    yc_ps = px_psum.tile([NP, 2048], F32, tag="yc", name=f"yc{st}")
                    nch = (ncols + 511) // 512
                    for mat in range(3):
                        for n in range(nch):
                            c0 = n * 512
                            c1 = min(c0 + 512, ncols)
                            if mat == 0:
                                rhs = xi_sb[:, col0 + c0 : col0 + c1]
                            elif mat == 1:
                                rhs = sq_t[:, c0:c1]
                            else:
                                rhs = cr_t[:, c0:c1]
                            nc.tensor.matmul(
                                yc_ps[:, c0:c1],
                                lhsT[:, mat, :],
                                rhs,
                                start=(mat == 0),
                                stop=(mat == 2),
                            )

                    y_sb = ring_pool.tile([NP, 2048], F16, tag="ysb", name=f"y{st}")
                    nc.scalar.copy(y_sb[:, 0:ncols], yc_ps[:, 0:ncols])
                    nc.sync.dma_start(out=y_d[:, sl], in_=y_sb[:, 0:ncols])
                    col0 += ncols

    nc.compile()
    return nc


def _prep_inputs(x, histogram, W1, b1, W2, b2):
    """Host-side sharding / layout packing (layout + dtype only; no data
    arithmetic)."""
    x = np.asarray(x, dtype=np.float32)
    hist = np.asarray(histogram, dtype=np.float32).reshape(B, HIST)
    W1 = np.asarray(W1, dtype=np.float32)
    b1 = np.asarray(b1, dtype=np.float32)
    W2 = np.asarray(W2, dtype=np.float32)
    b2 = np.asarray(b2, dtype=np.float32)

    w1pm = np.ascontiguousarray(
        W1.reshape(KT, P, HID).transpose(1, 0, 2)
    ).astype(np.float16)
    b1rep = np.ascontiguousarray(np.broadcast_to(b1, (SPC, HID)))
    e3 = np.zeros((SPC, 3, NP), dtype=np.float32)
    for i in range(3):
        for s in range(SPC):
            e3[s, i, 42 * i + G_S * s : 42 * i + G_S * (s + 1)] = 1.0

    # W2 / b2 with interleave-mapped columns
    cm = np.empty((3, 9), dtype=np.int64)  # [i, 3*mat+c]
    for i in range(3):
        for mat in range(3):
            for c in range(C):
                cm[i, 3 * mat + c] = _colmap(mat, i, c)
    w2i = np.ascontiguousarray(
        W2.reshape(MT, P, MOUT)[:, :, cm.reshape(-1)].reshape(MT, P, 3, 9)
        .reshape(MT, P, 27)
    )
    b2i = np.empty((NP, 9), dtype=np.float32)
    for i in range(3):
        b2i[42 * i : 42 * (i + 1), :] = b2[cm[i]]

    maskS = np.zeros((NP, G), dtype=np.float16)
    for i in range(3):
        for g in range(G):
            maskS[42 * i + g, g] = 1.0

    rot = np.concatenate(
        [np.arange(42, 84), np.arange(84, 126), np.arange(0, 42)]
    )

    in_maps = []
    for core in range(N_CORES):
        xI = np.zeros((NP, XCOLS), dtype=np.float16)
        for s in range(SPC):
            xs = x[core * SPC + s].reshape(PX_SAMPLE, C)
            pad = np.zeros((G_S * XCOLS, C), dtype=np.float32)
            pad[:PX_SAMPLE] = xs
            v = pad.reshape(G_S, XCOLS, C)  # [g', n, i]
            for i in range(3):
                xI[42 * i + G_S * s : 42 * i + G_S * (s + 1), :] = v[:, :, i].astype(
                    np.float16
                )
        xR = np.ascontiguousarray(xI[rot])

        h_core = hist[core * SPC : (core + 1) * SPC]
        hp = np.ascontiguousarray(
            h_core.reshape(SPC, KT, P).transpose(2, 1, 0).reshape(P, KT * SPC)
        ).astype(np.float16)
        in_maps.append(
            {
                "xi": xI,
                "xr": xR,
                "w1pm": w1pm,
                "h_packed": hp,
                "b1_rep": b1rep,
                "w2i": w2i,
                "b2i": b2i,
                "e3": e3,
                "maskS": maskS,
            }
        )
    return in_maps


def _unpack_output(res):
    y = np.empty((B, H, W, C), dtype=np.float32)
    for core in range(N_CORES):
        yb = np.asarray(res.results[core]["y_bands"])  # [126, XCOLS] f16
        for s in range(SPC):
            v = yb[:, :].reshape(3, G, XCOLS)[:, G_S * s : G_S * (s + 1), :]
            # v[c, g', n] -> pixel g'*XCOLS + n
            flat = v.transpose(1, 2, 0).reshape(G_S * XCOLS, C)[:PX_SAMPLE]
            y[core * SPC + s] = flat.reshape(H, W, C).astype(np.float32)
    return y


def run(trace=False, **inputs):
    if "nc" not in _CACHE:
        _CACHE["nc"] = _build()
    nc = _CACHE["nc"]
    in_maps = _prep_inputs(**inputs)
    res = bass_utils.run_bass_kernel_spmd(
        nc, in_maps, core_ids=list(range(N_CORES)), trace=trace
    )
    y = _unpack_output(res)
    return y, res


def kernel(**inputs) -> np.ndarray:
    y, _ = run(trace=False, **inputs)
    return y


if __name__ == "__main__":
    rng = np.random.default_rng(0)
    ins = {
        "x": rng.random((B, H, W, C), dtype=np.float32),
        "histogram": rng.random((B, 3, 64, 64), dtype=np.float32),
        "W1": (rng.standard_normal((HIST, HID)) / np.sqrt(HIST)).astype(np.float32),
        "b1": np.zeros(HID, np.float32),
        "W2": (rng.standard_normal((HID, MOUT)) / np.sqrt(HID)).astype(np.float32),
        "b2": np.zeros(MOUT, np.float32),
    }
    y = kernel(**ins)
    print("out", y.shape, y.dtype, float(np.abs(y).max()))
